# revision 3
# baseline (speedup 1.0000x reference)
"""Catmull-Rom spline loss kernel for Trainium2 (8 NeuronCores, SPMD).

loss = sum((ch1 - mapped)^2), mapped[n,c] = sum_{k,q} wx[n,k] wy[n,q]
CP[i-1+k, j-1+q, c].

Strategy: gather-free "cell slots" for ~94% of points, one overlapped
dma_gather for the rest.
  - Each partition p holds grid rows 4p-1..4p+5 in SBUF (fp16 window W).
  - Cell passes: one slot per grid cell (i,j): partition i>>2, free
    index j (per delta=i&3).  Host routes each point's global rank r
    within its cell to a core; pass 0 covers ranks 0..7 (all 4 deltas),
    pass 1 covers ranks 8..15 for cells with i%4 < P2_DELTAS.  Patch
    reads are overlapping strided APs on W -- no gather.  Empty slots
    get x=y=0 and c01=grid16[i,j] so they contribute exactly 0.
  - Weight tensors are expanded ([s] -> [s,8]) on the Scalar engine so
    the bilinear contraction runs in DVE 2x packed mode.
  - Tier-2 (the rest, ~2.4% of points): dma_gather of 256B entries from
    a device-built banded table GB[(r,b)] = rows r-1..r+2 x colch
    16b..16b+31, phase-grouped by j%8; generation overlaps the passes.
  - Squared-diff reduction on the Scalar engine (Square + accum_out).

Host work: dtype casts, permutation/routing, padding, int16 index
packing only.
"""

import sys

for _p in ("/opt/trn_rl_repo",):
    if _p not in sys.path:
        sys.path.insert(0, _p)

from contextlib import ExitStack

import numpy as np

from concourse import bacc, bass, mybir, tile
from concourse.ap import AP
from concourse.bass_utils import run_bass_kernel_spmd

F32 = mybir.dt.float32
F16 = mybir.dt.float16
I16 = mybir.dt.int16
OP = mybir.AluOpType
AF = mybir.ActivationFunctionType

G = 512
N_CORES = 8
WROW = 1040            # padded fp16 window row: colch 2..1025 hold the grid
P2_DELTAS = 2          # pass-1 covers cells with i%4 < P2_DELTAS
NDELTA = 4 + P2_DELTAS
T2_PER_PHI = 2304      # tier-2 slots per phase group (per core)
T2_SLOTS = 8 * T2_PER_PHI          # 18432
T2F = T2_SLOTS // 128              # 144
T2H = T2_PER_PHI // 128            # 18 free slots per phase group
T2_CHUNK = 2 * T2_PER_PHI          # 4608 idxs per gather (2 phase groups)
T2_NCHUNK = 4


def _wap(t, off_elems, dims):
    """Manual free-dim AP on tile t: dims = [(stride, size), ...]."""
    a = t[:]
    return AP(tensor=a.tensor, offset=a.offset + off_elems,
              ap=[list(a.ap[0])] + [list(d) for d in dims])


def build_nc():
    nc = bacc.Bacc("TRN2", target_bir_lowering=False, debug=False,
                   dynamic_dma_scratch_size=20480, num_swdge_queues=4)

    cp = nc.dram_tensor("cp", [G, 2 * G], F32, kind="ExternalInput")
    nslot = 2048 + P2_DELTAS * 512
    xs = nc.dram_tensor("xs", [128, nslot], F16, kind="ExternalInput")
    ys = nc.dram_tensor("ys", [128, nslot], F16, kind="ExternalInput")
    c01 = nc.dram_tensor("c01", [128, nslot, 2], F16, kind="ExternalInput")
    xy2 = nc.dram_tensor("xy2", [128, 2 * T2F], F16, kind="ExternalInput")
    c012 = nc.dram_tensor("c012", [128, T2F, 2], F16, kind="ExternalInput")
    gidx = nc.dram_tensor("gidx", [T2_NCHUNK, 128, T2_CHUNK // 16], I16,
                          kind="ExternalInput")
    out = nc.dram_tensor("out", [128, 1], F32, kind="ExternalOutput")

    gb = nc.dram_tensor("gb", [G * 64, 128], F16)   # banded patch table
    cp_ap = cp.ap()

    with tile.TileContext(nc) as tc, ExitStack() as ctx:
        w_pool = ctx.enter_context(tc.tile_pool(name="w", bufs=1))
        acc_pool = ctx.enter_context(tc.tile_pool(name="acc", bufs=1))
        g_pool = ctx.enter_context(tc.tile_pool(name="g", bufs=1))

        W = w_pool.tile([128, 7, WROW], F16, name="W")
        acc = acc_pool.tile([128, 8], F32, name="acc")
        nb = acc_pool.tile([128, 1], F32, name="nb")
        nc.vector.memset(nb[:], -1.0)

        # ---- load grid rows 4p-1..4p+5 as fp32, cast to fp16 window --
        with ExitStack() as bctx:
            wf_pool = bctx.enter_context(tc.tile_pool(name="wf", bufs=1))
            Wf = wf_pool.tile([128, 7, 1024], F32, name="Wf")
            nc.gpsimd.memset(_wap(W, 0, [(WROW, 7), (1, 2)]), 0.0)
            nc.gpsimd.memset(_wap(W, 1026, [(WROW, 7), (1, 14)]), 0.0)
            nc.sync.dma_start(
                out=Wf[1:127, :, :],
                in_=AP(tensor=cp_ap.tensor, offset=3 * 1024,
                       ap=[[4096, 126], [1024, 7], [1, 1024]]),
            )
            nc.sync.dma_start(out=Wf[0:1, 0:1, :], in_=cp_ap[0:1, :])
            nc.sync.dma_start(
                out=Wf[0:1, 1:7, :],
                in_=AP(tensor=cp_ap.tensor, offset=0,
                       ap=[[6144, 1], [1024, 6], [1, 1024]]),
            )
            nc.sync.dma_start(
                out=Wf[127:128, 0:5, :],
                in_=AP(tensor=cp_ap.tensor, offset=507 * 1024,
                       ap=[[5120, 1], [1024, 5], [1, 1024]]),
            )
            nc.sync.dma_start(
                out=Wf[127:128, 5:7, :],
                in_=AP(tensor=cp_ap.tensor, offset=510 * 1024,
                       ap=[[2048, 1], [1024, 2], [1, 1024]]),
            )
            nc.scalar.activation(
                _wap(W, 2, [(WROW, 7), (1, 1024)]), Wf[:], AF.Copy)

        # ---- GB table + tier-2 gathers (gen overlaps the passes) -----
        with ExitStack() as bctx:
            gbp = bctx.enter_context(tc.tile_pool(name="gb", bufs=2))
            gb_flat = gb.ap().rearrange("(p x) e -> p (x e)", p=128)
            for d in range(4):
                gb_t = gbp.tile([128, 64, 128], F16, tag="gbt", name="gbt")
                nc.vector.tensor_copy(
                    out=gb_t[:].rearrange("p b (k c) -> p b k c", k=4),
                    in_=_wap(W, d * WROW, [(16, 64), (WROW, 4), (1, 32)]),
                )
                nc.sync.dma_start(
                    out=gb_flat[:, d * 8192:(d + 1) * 8192],
                    in_=gb_t[:].rearrange("p b e -> p (b e)"),
                )
        t2in = g_pool.tile([128, T2F, 4, 8], F16, name="t2in")
        gx_pool = ctx.enter_context(tc.tile_pool(name="gx", bufs=1))
        mg_pool = ctx.enter_context(tc.tile_pool(name="mg", bufs=2))
        gx_ts = []
        for c in range(T2_NCHUNK):
            gx_t = gx_pool.tile([128, T2_CHUNK // 16], I16, tag=f"gx{c}",
                                name="gx_t")
            nc.sync.dma_start(out=gx_t[:], in_=gidx.ap()[c])
            gx_ts.append(gx_t)
        for c in range(T2_NCHUNK):
            gx_t = gx_ts[c]
            mega = mg_pool.tile([128, 2 * T2H, 128], F16, tag="mega",
                                name="mega")
            nc.gpsimd.dma_gather(mega[:], gb.ap(), gx_t[:], T2_CHUNK,
                                 T2_CHUNK, 128, single_packet=False,
                                 queue_num=c % 4)
            # align-copies on gpsimd: serialized after the gather there,
            # cannot stall the vector stream.
            for h in range(2):
                ph = 2 * c + h
                nc.gpsimd.tensor_copy(
                    out=t2in[:, T2H * ph: T2H * (ph + 1)],
                    in_=_wap(mega, (T2H * h) * 128 + 2 * ph,
                             [(128, T2H), (32, 4), (1, 8)]),
                )

        # ---- weight computation helper -------------------------------
        def weights(vt, wpool, n, tag, stag=""):
            """vt: [128, n] f16 values in [0,1). Returns w0..w3 tiles."""
            V = vt[:]
            xm = wpool.tile([128, n], F16, tag=f"xm{stag}", name="xm")
            x2 = wpool.tile([128, n], F16, tag=f"x2{stag}", name="x2")
            t1 = wpool.tile([128, n], F16, tag=f"t1{stag}", name="t1")
            e = wpool.tile([128, n], F16, tag=f"e{stag}", name="e")
            w = [wpool.tile([128, n], F16, tag=f"w{k}{tag}",
                            name=f"w{k}{tag}") for k in range(4)]
            nc.scalar.activation(xm[:], V, AF.Copy, bias=-1.0)
            nc.scalar.activation(x2[:], V, AF.Square)
            nc.scalar.activation(t1[:], V, AF.Square, bias=nb[:])
            nc.scalar.activation(e[:], V, AF.Copy, bias=-2.5, scale=1.5)
            nc.vector.scalar_tensor_tensor(w[0][:], V, -0.5, t1[:],
                                           OP.mult, OP.mult)
            nc.vector.scalar_tensor_tensor(w[3][:], x2[:], 0.5, xm[:],
                                           OP.mult, OP.mult)
            nc.vector.tensor_tensor(t1[:], e[:], x2[:], OP.mult)
            nc.vector.tensor_scalar(w[1][:], t1[:], 1.0, None, OP.add)
            nc.vector.tensor_tensor(t1[:], w[1][:], w[0][:], OP.add)
            nc.vector.tensor_tensor(t1[:], t1[:], w[3][:], OP.add)
            nc.scalar.activation(w[2][:], t1[:], AF.Copy, bias=1.0,
                                 scale=-1.0)
            return w

        # ---- cell passes (vector + scalar only) ----------------------
        s_pool = ctx.enter_context(tc.tile_pool(name="s", bufs=1))
        wt_pool = ctx.enter_context(tc.tile_pool(name="wt", bufs=1))
        we_pool = ctx.enter_context(tc.tile_pool(name="we", bufs=1))
        t_pool = ctx.enter_context(tc.tile_pool(name="t", bufs=1))

        xy2t = g_pool.tile([128, 2 * T2F], F16, name="xy2t")
        c012t = g_pool.tile([128, T2F, 2], F16, name="c012t")
        nc.sync.dma_start(out=xy2t[:], in_=xy2.ap()[:, :])
        nc.sync.dma_start(out=c012t[:], in_=c012.ap()[:, :])

        # preload every stream tile up front: no DMA issues mid-kernel,
        # so nothing can queue behind the gather pipeline on any engine.
        xyts, c01ts = [], []
        for it in range(NDELTA):
            sl = slice(512 * it, 512 * (it + 1))
            xyt = s_pool.tile([128, 1024], F16, tag=f"xyt{it}", name="xyt")
            c01t = s_pool.tile([128, 512, 2], F16, tag=f"c01t{it}",
                               name="c01t")
            nc.sync.dma_start(out=xyt[:, 0:512], in_=xs.ap()[:, sl])
            nc.sync.dma_start(out=xyt[:, 512:1024], in_=ys.ap()[:, sl])
            nc.sync.dma_start(out=c01t[:], in_=c01.ap()[:, sl])
            xyts.append(xyt)
            c01ts.append(c01t)

        for it in range(NDELTA):
            d = it if it < 4 else it - 4
            xyt, c01t = xyts[it], c01ts[it]
            w = weights(xyt, wt_pool, 1024, f"a{it % 2}")
            # expand wx_k -> [s,8] and wy -> interleaved [s,(q,c)] on ACT
            wxe = []
            for k in range(4):
                wk = we_pool.tile([128, 512, 8], F16,
                                  tag=f"wxe{k}_{it % 2}", name="wk")
                nc.scalar.activation(
                    wk[:], w[k][:, 0:512].to_broadcast([128, 512, 8]),
                    AF.Copy)
                wxe.append(wk)
            wyi = we_pool.tile([128, 512, 8], F16, tag=f"wyi{it % 2}",
                               name="wyi")
            for q in range(4):
                nc.scalar.activation(
                    _wap(wyi, 2 * q, [(8, 512), (1, 2)]),
                    w[q][:, 512:1024].to_broadcast([128, 512, 2]),
                    AF.Copy)
            T = t_pool.tile([128, 512, 8], F16, tag="T", name="T")
            tmp = t_pool.tile([128, 512, 8], F16, tag="tmp", name="tmp")
            for k in range(4):
                pk = _wap(W, (d + k) * WROW, [(2, 512), (1, 8)])
                if k == 0:
                    nc.vector.tensor_tensor(T[:], pk, wxe[0][:], OP.mult)
                else:
                    nc.vector.tensor_tensor(tmp[:], pk, wxe[k][:], OP.mult)
                    nc.vector.tensor_tensor(T[:], T[:], tmp[:], OP.add)
            nc.vector.tensor_tensor(tmp[:], T[:], wyi[:], OP.mult)
            mm = t_pool.tile([128, 512, 2], F16, tag="mm", name="mm")
            mtv = _wap(tmp, 2, [(8, 512), (1, 2)])
            nc.vector.tensor_tensor(mm[:], tmp[:, :, 0:2], tmp[:, :, 2:4],
                                    OP.add)
            nc.vector.tensor_tensor(mtv, tmp[:, :, 4:6], tmp[:, :, 6:8],
                                    OP.add)
            nc.vector.tensor_tensor(mm[:], mm[:], mtv, OP.add)
            dt = t_pool.tile([128, 512, 2], F16, tag="dt", name="dt")
            sq = t_pool.tile([128, 512, 2], F16, tag="sq", name="sq")
            nc.vector.tensor_tensor(dt[:], mm[:], c01t[:], OP.subtract)
            nc.scalar.activation(sq[:], dt[:], AF.Square,
                                 accum_out=acc[:, it:it + 1])

        # ---- tier-2 compute (after the passes; reuses pass pools) ----
        t2_pool = t_pool
        w2l = weights(xy2t, wt_pool, 2 * T2F, "b")
        wx2 = [t[:, 0:T2F] for t in w2l]
        wy2 = [t[:, T2F:2 * T2F] for t in w2l]
        T2 = t2_pool.tile([128, T2F, 8], F16, tag="T", name="T2")
        tm2 = t2_pool.tile([128, T2F, 8], F16, tag="tmp", name="tm2")
        for k in range(4):
            pk = t2in[:, :, k, :]
            wb = wx2[k].to_broadcast([128, T2F, 8])
            if k == 0:
                nc.vector.tensor_tensor(T2[:], pk, wb, OP.mult)
            else:
                nc.vector.tensor_tensor(tm2[:], pk, wb, OP.mult)
                nc.vector.tensor_tensor(T2[:], T2[:], tm2[:], OP.add)
        m2 = t2_pool.tile([128, T2F, 2], F16, tag="mm", name="m2")
        m2t = t2_pool.tile([128, T2F, 2], F16, tag="dt", name="m2t")
        for q in range(4):
            tq = T2[:, :, 2 * q: 2 * q + 2]
            wb = wy2[q].to_broadcast([128, T2F, 2])
            if q == 0:
                nc.vector.tensor_tensor(m2[:], tq, wb, OP.mult)
            else:
                nc.vector.tensor_tensor(m2t[:], tq, wb, OP.mult)
                nc.vector.tensor_tensor(m2[:], m2[:], m2t[:], OP.add)
        d2 = wt_pool.tile([128, T2F, 2], F16, tag="d2b", name="d2")
        sq2 = t2_pool.tile([128, T2F, 2], F16, tag="sq", name="sq2")
        nc.vector.tensor_tensor(d2[:], m2[:], c012t[:], OP.subtract)
        nc.scalar.activation(sq2[:], d2[:], AF.Square,
                             accum_out=acc[:, NDELTA:NDELTA + 1])

        # ---- final reduce --------------------------------------------
        fin = acc_pool.tile([128, 1], F32, name="fin")
        nc.vector.tensor_reduce(fin[:], acc[:, 0:NDELTA + 1],
                                mybir.AxisListType.X, OP.add)
        nc.sync.dma_start(out=out.ap()[:, :], in_=fin[:])

    nc.compile()
    return nc


def host_prep(ch1, CP_locs, CP_idx, r, n_cores=N_CORES):
    N = ch1.shape[0]
    i64 = CP_idx[:, 0].astype(np.int64)
    j64 = CP_idx[:, 1].astype(np.int64)
    x = (np.asarray(r[:, 0], np.float32) % 1.0).astype(np.float16)
    y = (np.asarray(r[:, 1], np.float32) % 1.0).astype(np.float16)
    c1 = np.asarray(ch1, np.float32).astype(np.float16)      # [N,2]
    cp_f = np.ascontiguousarray(
        np.asarray(CP_locs, np.float32).reshape(G, 2 * G))
    grid16 = np.asarray(CP_locs, np.float32).astype(np.float16)  # [G,G,2]

    cell = i64 * G + j64
    order = np.argsort(cell, kind="stable")
    cs = cell[order]
    first = np.empty(N, bool)
    first[0] = True
    first[1:] = cs[1:] != cs[:-1]
    fidx = np.where(first, np.arange(N), 0)
    np.maximum.accumulate(fidx, out=fidx)
    rank = np.arange(N) - fidx
    ii_s, jj_s = i64[order], j64[order]
    hc = ((ii_s >> 2) + 5 * (jj_s >> 3) + 3 * (jj_s & 7)) % n_cores
    core = np.where(rank < n_cores, rank % n_cores, (rank + hc) % n_cores)
    dlt = (ii_s % 4)
    pass1 = (rank >= n_cores) & (rank < 2 * n_cores) & (dlt < P2_DELTAS)
    pass0 = rank < n_cores
    tier2 = ~(pass0 | pass1)

    nslot = 2048 + P2_DELTAS * 512
    base_c01 = grid16.reshape(G * G, 2)
    g4 = grid16.reshape(128, 4, G, 2)
    base_c01_p1 = g4[:, :P2_DELTAS].reshape(128 * P2_DELTAS * G, 2)

    in_maps = []
    for c in range(n_cores):
        xs = np.zeros((128, nslot), np.float16)
        ysa = np.zeros((128, nslot), np.float16)
        c01 = np.empty((128, nslot, 2), np.float16)
        c01[:, 0:2048] = base_c01.reshape(128, 2048, 2)
        c01[:, 2048:] = base_c01_p1.reshape(128, P2_DELTAS * G, 2)

        sel0 = pass0 & (core == c)
        p0 = order[sel0]
        flat0 = cs[sel0]
        xs.reshape(-1)[(flat0 // 2048) * nslot + flat0 % 2048] = x[p0]
        ysa.reshape(-1)[(flat0 // 2048) * nslot + flat0 % 2048] = y[p0]
        c01.reshape(-1, 2)[(flat0 // 2048) * nslot + flat0 % 2048] = c1[p0]

        sel1 = pass1 & (core == c)
        p1 = order[sel1]
        i1, j1 = i64[p1], j64[p1]
        flat1 = (i1 >> 2) * nslot + 2048 + (i1 & 3) * 512 + j1
        xs.reshape(-1)[flat1] = x[p1]
        ysa.reshape(-1)[flat1] = y[p1]
        c01.reshape(-1, 2)[flat1] = c1[p1]

        sel2 = tier2 & (core == c)
        p2 = order[sel2]
        i2, j2 = i64[p2], j64[p2]
        phi2 = j2 & 7
        ent2 = (i2 * 64 + (j2 >> 3)).astype(np.int16)
        x2a = np.zeros(T2_SLOTS, np.float16)
        y2a = np.zeros(T2_SLOTS, np.float16)
        e2a = np.full(T2_SLOTS, 64, np.int16)          # pad: entry (1, 0)
        c2a = np.zeros((T2_SLOTS, 2), np.float16)
        for ph in range(8):
            s0 = ph * T2_PER_PHI
            c2a[s0:s0 + T2_PER_PHI] = grid16[1, ph]    # pad c01
            m = phi2 == ph
            n = int(m.sum())
            assert n <= T2_PER_PHI, f"tier2 overflow: core {c} phi {ph} {n}"
            x2a[s0:s0 + n] = x[p2[m]]
            y2a[s0:s0 + n] = y[p2[m]]
            e2a[s0:s0 + n] = ent2[m]
            c2a[s0:s0 + n] = c1[p2[m]]

        x2_dev = np.ascontiguousarray(x2a.reshape(T2F, 128).T)
        y2_dev = np.ascontiguousarray(y2a.reshape(T2F, 128).T)
        xy2_dev = np.concatenate([x2_dev, y2_dev], axis=1)
        c012_dev = np.ascontiguousarray(
            np.moveaxis(c2a.reshape(T2F, 128, 2), 1, 0))
        gidx_dev = np.zeros((T2_NCHUNK, 128, T2_CHUNK // 16), np.int16)
        for ch in range(T2_NCHUNK):
            w16 = e2a[ch * T2_CHUNK:(ch + 1) * T2_CHUNK].reshape(
                T2_CHUNK // 16, 16).T
            gidx_dev[ch] = np.tile(w16, (8, 1))
        in_maps.append({
            "cp": cp_f, "xs": xs, "ys": ysa, "c01": c01,
            "xy2": xy2_dev, "c012": c012_dev, "gidx": gidx_dev,
        })
    return in_maps


_NC_CACHE = {}


def kernel(ch1, CP_locs, CP_idx, r):
    ch1, CP_locs = np.asarray(ch1), np.asarray(CP_locs)
    CP_idx, r = np.asarray(CP_idx), np.asarray(r)
    if "nc" not in _NC_CACHE:
        _NC_CACHE["nc"] = build_nc()
    nc = _NC_CACHE["nc"]
    in_maps = host_prep(ch1, CP_locs, CP_idx, r)
    res = run_bass_kernel_spmd(nc, in_maps, list(range(N_CORES)))
    total = np.float64(0.0)
    for rmap in res.results:
        total += np.float64(rmap["out"]).sum()
    return np.array(total, dtype=np.float32)
